# revision 8
# baseline (speedup 1.0000x reference)
"""Trainium2 Bass kernel: cross-entropy with Gaussian-smoothed labels.

loss = mean over tokens of  [ Wsum(t) * logsumexp(pred_row) - sum_k w_k * pred[start+k] ]

where the smoothed one-hot reduces exactly to a 7-tap window:
  start = clip(t-3, 0, C-7), u = t-start, w_k = f(k-u)
  f(0)=1.0, f(+-m)=exp(-2^m/4) for m in 1..3, else 0.

Sharding: pure data-parallel over the batch axis, 4 batches (8192 tokens)
per core across 8 cores. Per core:
  - stream pred [8192, 722] f32 through SBUF in [128, G*722] tiles,
    exp on ACT; per-token sum(exp) via mixed ACT-accum / DVE segmented
    reduce (balances the two engines); Ln -> lse.
  - window gathers batched into a few big indirect DMAs (one SWDGE call
    covers many tokens; per-call fixed cost ~1us amortized away).
  - weights built once on-chip from |k-u| via two chained Exp ops.
  - per-core partial sums [128, 1] DMA'd out; host sums 8x128 and divides.
"""
import json
import math
import os

import numpy as np

import concourse.bass as bass
import concourse.bacc as bacc
import concourse.tile as tile
from concourse import mybir
from concourse import bass_utils

B, T, C = 32, 2048, 722
CORES = 8
SHARD = B * T // CORES          # 8192 tokens per core
P = 128
TILES = SHARD // P              # 64 token-tiles of 128 tokens
K = 7
START_MAX = C - K               # 715
LN2 = math.log(2.0)
D0_FIX = 1.0 - math.exp(-0.25)  # lift f(0) from exp(-2^0/4) to 1.0

_ALU = mybir.AluOpType
_ACT = mybir.ActivationFunctionType

_NC = None


def _bcast_inner(ap, n):
    """Append a step-0 broadcast dim of length n to an AP."""
    return bass.AP(tensor=ap.tensor, offset=ap.offset, ap=[*ap.ap, [0, n]])


# cubic through f(0..3) = 1, e^-.5, e^-1, e^-2; exact at integer distances
_C3 = -0.024785177547111593
_C2 = 0.15176460742141516
_C1 = -0.5204487670682929


def _build(G=4, gsplit=64, naccum=16, ndve=6, pred_bufs=6, exp_bufs=3,
           two_ring=False, fastw=True, asserts=False, dump=False):
    NG = TILES // G
    # which DMA groups compute sum(exp) via ACT accumulate (the rest use one
    # big ACT exp + a segmented DVE reduce) -- spread evenly over the stream
    accum_set = set() if naccum == 0 else \
        {min(NG - 1, int((i + 0.5) * NG / naccum)) for i in range(naccum)}
    # token-tiles whose window-sum is computed densely on DVE (no gather)
    dve_set = set() if ndve == 0 else \
        {min(TILES - 1, int((i + 0.5) * TILES / ndve)) for i in range(ndve)}

    nc = bacc.Bacc("TRN2", target_bir_lowering=False, debug=False,
                   enable_asserts=asserts, num_devices=CORES)
    pred = nc.dram_tensor("pred", [SHARD, C], mybir.dt.float32, kind="ExternalInput")
    target = nc.dram_tensor("target", [SHARD], mybir.dt.int32, kind="ExternalInput")
    out = nc.dram_tensor("partial", [P, 1], mybir.dt.float32, kind="ExternalOutput")
    dumps = {}
    if dump:
        for name, shape in (("d_sums", [P, TILES]), ("d_wsum", [P, TILES]),
                            ("d_lse", [P, TILES]), ("d_gsum", [P, TILES]),
                            ("d_w", [P, TILES, K]), ("d_gath", [P, TILES, K])):
            dumps[name] = nc.dram_tensor(name, shape, mybir.dt.float32,
                                         kind="ExternalOutput")

    pred_flat = pred.ap().rearrange("a b -> (a b)").rearrange("(n one) -> n one", one=1)
    # token index = p*TILES + jg*G + g  (each partition owns a contiguous slab)
    pred_g = pred.ap().rearrange("(p j g) c -> j p g c", p=P, g=G)

    with tile.TileContext(nc) as tc:
        with (tc.tile_pool(name="pred", bufs=pred_bufs) as pred_pool,
              tc.tile_pool(name="exp", bufs=exp_bufs) as exp_pool,
              tc.tile_pool(name="small", bufs=1) as small):
            # targets: tgt_sb[p, j] = target[p*TILES + j]
            tgt_sb = small.tile([P, TILES], mybir.dt.int32)
            nc.sync.dma_start(out=tgt_sb, in_=target.ap().rearrange("(p j) -> p j", p=P))

            # flat element offsets of each token's window start
            row = small.tile([P, TILES], mybir.dt.int32)
            nc.gpsimd.iota(row, pattern=[[1, TILES]], base=0, channel_multiplier=TILES)
            start_i = small.tile([P, TILES], mybir.dt.int32)
            nc.vector.tensor_scalar(out=start_i, in0=tgt_sb, scalar1=3, scalar2=0,
                                    op0=_ALU.subtract, op1=_ALU.max)
            nc.vector.tensor_scalar_min(out=start_i, in0=start_i, scalar1=START_MAX)
            offs = small.tile([P, TILES], mybir.dt.int32)
            nc.vector.tensor_scalar_mul(out=offs, in0=row, scalar1=C)
            nc.vector.tensor_add(out=offs, in0=offs, in1=start_i)

            # u = t - start (0..6); diff[p,j,k] = k - u[p,j]
            ui = small.tile([P, TILES], mybir.dt.int32)
            nc.vector.tensor_sub(out=ui, in0=tgt_sb, in1=start_i)
            uf = small.tile([P, TILES], mybir.dt.float32)
            nc.vector.tensor_copy(out=uf, in_=ui)

            iok = small.tile([P, TILES, K], mybir.dt.float32)
            nc.gpsimd.iota(iok, pattern=[[0, TILES], [1, K]], base=0,
                           channel_multiplier=0, allow_small_or_imprecise_dtypes=True)
            diff = small.tile([P, TILES, K], mybir.dt.float32)
            nc.vector.scalar_tensor_tensor(out=diff, in0=iok, scalar=1.0,
                                           in1=_bcast_inner(uf, K),
                                           op0=_ALU.mult, op1=_ALU.subtract)
            w = small.tile([P, TILES, K], mybir.dt.float32)
            if fastw:
                # w = exp(-2^|d|/4) * (|d|<=3) + (|d|==0)*(1-exp(-1/4))
                ad = small.tile([P, TILES, K], mybir.dt.float32)
                nc.vector.scalar_tensor_tensor(out=ad, in0=diff, scalar=-1.0,
                                               in1=diff, op0=_ALU.mult, op1=_ALU.max)
                p2 = small.tile([P, TILES, K], mybir.dt.float32)
                nc.scalar.activation(out=p2, in_=ad, func=_ACT.Exp, scale=LN2)
                wu = small.tile([P, TILES, K], mybir.dt.float32)
                nc.scalar.activation(out=wu, in_=p2, func=_ACT.Exp, scale=-0.25)
                msk = small.tile([P, TILES, K], mybir.dt.float32)
                nc.vector.tensor_scalar(out=msk, in0=ad, scalar1=3.0, scalar2=None,
                                        op0=_ALU.is_le)
                eq0 = small.tile([P, TILES, K], mybir.dt.float32)
                nc.vector.tensor_scalar(out=eq0, in0=ad, scalar1=0.0, scalar2=None,
                                        op0=_ALU.is_equal)
                nc.vector.tensor_mul(out=w, in0=wu, in1=msk)
                nc.vector.scalar_tensor_tensor(out=w, in0=eq0, scalar=D0_FIX,
                                               in1=w, op0=_ALU.mult, op1=_ALU.add)
            else:
                DECAYS = [math.exp(-(2.0 ** d) / 4.0) for d in range(4)]
                nc.vector.tensor_scalar(out=w, in0=diff, scalar1=0.0, scalar2=None,
                                        op0=_ALU.is_equal)
                tmp = small.tile([P, TILES, K], mybir.dt.float32)
                for m in (1, 2, 3):
                    for s in (-m, m):
                        nc.vector.tensor_scalar(out=tmp, in0=diff, scalar1=float(s),
                                                scalar2=None, op0=_ALU.is_equal)
                        nc.vector.scalar_tensor_tensor(out=w, in0=tmp, scalar=DECAYS[m],
                                                       in1=w, op0=_ALU.mult, op1=_ALU.add)
            wsum = small.tile([P, TILES], mybir.dt.float32)
            nc.vector.reduce_sum(out=wsum, in_=w, axis=mybir.AxisListType.X)

            # windowed gathers: one indirect DMA per token-tile ([P,1] offsets
            # is the only form the HW SWDGE lowers correctly)
            gath = small.tile([P, TILES, K], mybir.dt.float32)
            if gsplit == 64:
                for j in range(TILES):
                    if j in dve_set:
                        nc.vector.memset(gath[:, j, :], 0.0)
                        continue
                    nc.gpsimd.indirect_dma_start(
                        out=gath[:, j, :],
                        out_offset=None,
                        in_=pred_flat,
                        in_offset=bass.IndirectOffsetOnAxis(
                            ap=offs[:, j:j + 1], axis=0),
                    )
            else:
                cols = TILES // gsplit
                for s in range(gsplit):
                    j0 = s * cols
                    nc.gpsimd.indirect_dma_start(
                        out=gath[:, j0:j0 + cols, :],
                        out_offset=None,
                        in_=pred_flat,
                        in_offset=bass.IndirectOffsetOnAxis(
                            ap=offs[:, j0:j0 + cols], axis=0),
                    )

            # prep for dense (gather-free) window sums on DVE
            gsum_d = small.tile([P, TILES], mybir.dt.float32)
            if dve_set:
                nc.vector.memset(gsum_d, 0.0)
                tf = small.tile([P, TILES], mybir.dt.float32)
                nc.vector.tensor_copy(out=tf, in_=tgt_sb)
                iota722 = small.tile([P, C], mybir.dt.float32)
                nc.gpsimd.iota(iota722, pattern=[[1, C]], base=0,
                               channel_multiplier=0,
                               allow_small_or_imprecise_dtypes=True)
                diffd = small.tile([P, C], mybir.dt.float32)
                add = small.tile([P, C], mybir.dt.float32)
                mskd = small.tile([P, C], mybir.dt.float32)
                t1d = small.tile([P, C], mybir.dt.float32)
                wdd = small.tile([P, C], mybir.dt.float32)
                wgd = small.tile([P, C], mybir.dt.float32)

            # dense stream: exp -> per-token sum(exp)
            sums = small.tile([P, TILES], mybir.dt.float32)
            for jg in range(NG):
                pt = pred_pool.tile([P, G, C], mybir.dt.float32)
                dma_eng = nc.scalar if (two_ring and jg % 2) else nc.sync
                dma_eng.dma_start(out=pt, in_=pred_g[jg])
                et = exp_pool.tile([P, G, C], mybir.dt.float32)
                if jg in accum_set:
                    for g in range(G):
                        j = jg * G + g
                        nc.scalar.activation(out=et[:, g, :], in_=pt[:, g, :],
                                             func=_ACT.Exp,
                                             accum_out=sums[:, j:j + 1])
                else:
                    nc.scalar.activation(out=et, in_=pt, func=_ACT.Exp)
                    nc.vector.reduce_sum(out=sums[:, jg * G:(jg + 1) * G], in_=et,
                                         axis=mybir.AxisListType.X)
                for g in range(G):
                    j = jg * G + g
                    if j not in dve_set:
                        continue
                    # dense W row: w(c) = cubic(|c - t|) * (|c - t| <= 3)
                    nc.vector.tensor_scalar(out=diffd, in0=iota722,
                                            scalar1=tf[:, j:j + 1], scalar2=None,
                                            op0=_ALU.subtract)
                    nc.vector.scalar_tensor_tensor(out=add, in0=diffd, scalar=-1.0,
                                                   in1=diffd, op0=_ALU.mult,
                                                   op1=_ALU.max)
                    nc.vector.tensor_scalar(out=mskd, in0=add, scalar1=3.0,
                                            scalar2=None, op0=_ALU.is_le)
                    nc.vector.tensor_scalar(out=t1d, in0=add, scalar1=_C3,
                                            scalar2=_C2, op0=_ALU.mult, op1=_ALU.add)
                    nc.vector.scalar_tensor_tensor(out=t1d, in0=t1d, scalar=0.0,
                                                   in1=add, op0=_ALU.add,
                                                   op1=_ALU.mult)
                    nc.vector.scalar_tensor_tensor(out=t1d, in0=t1d, scalar=_C1,
                                                   in1=add, op0=_ALU.add,
                                                   op1=_ALU.mult)
                    nc.vector.scalar_tensor_tensor(out=wdd, in0=t1d, scalar=1.0,
                                                   in1=mskd, op0=_ALU.add,
                                                   op1=_ALU.mult)
                    nc.vector.affine_mul_reduce(out=wgd,
                                                accum_out=gsum_d[:, j:j + 1],
                                                in0=wdd, in1=pt[:, g, :],
                                                scale=1.0, bias=0.0)

            # lse, weighted gather sums, per-core partial
            lse = small.tile([P, TILES], mybir.dt.float32)
            nc.scalar.activation(out=lse, in_=sums, func=_ACT.Ln)
            wg = small.tile([P, TILES, K], mybir.dt.float32)
            gsum = small.tile([P, TILES], mybir.dt.float32)
            nc.vector.tensor_mul(out=wg, in0=w, in1=gath)
            nc.vector.reduce_sum(out=gsum, in_=wg, axis=mybir.AxisListType.X)
            if dve_set:
                nc.vector.tensor_add(out=gsum, in0=gsum, in1=gsum_d)
            loss = small.tile([P, TILES], mybir.dt.float32)
            nc.vector.tensor_mul(out=loss, in0=wsum, in1=lse)
            nc.vector.tensor_sub(out=loss, in0=loss, in1=gsum)
            part = small.tile([P, 1], mybir.dt.float32)
            nc.vector.reduce_sum(out=part, in_=loss, axis=mybir.AxisListType.X)
            nc.sync.dma_start(out=out.ap(), in_=part)
            if dump:
                for name, t in (("d_sums", sums), ("d_wsum", wsum), ("d_lse", lse),
                                ("d_gsum", gsum), ("d_w", w), ("d_gath", gath)):
                    nc.sync.dma_start(out=dumps[name].ap(), in_=t)
    nc.compile()
    return nc


def _get_nc():
    global _NC
    if _NC is None:
        _NC = _build(**json.loads(os.environ.get("CEK_KNOBS", "{}")))
    return _NC


def _shard_inputs(pred, target):
    bpc = B // CORES
    in_maps = []
    for c in range(CORES):
        in_maps.append({
            "pred": np.ascontiguousarray(
                pred[c * bpc:(c + 1) * bpc].reshape(SHARD, C), dtype=np.float32),
            "target": np.ascontiguousarray(
                target[c * bpc:(c + 1) * bpc].reshape(SHARD), dtype=np.int32),
        })
    return in_maps


def _run(pred, target, **kwargs):
    nc = _get_nc()
    return bass_utils.run_bass_kernel_spmd(
        nc, _shard_inputs(pred, target), core_ids=list(range(CORES)), **kwargs)


def kernel(pred, target):
    res = _run(pred, target)
    total = sum(float(r["partial"].astype(np.float64).sum()) for r in res.results)
    return np.asarray(total / (B * T), dtype=np.float32)
